# revision 52
# baseline (speedup 1.0000x reference)
"""Causal self-attention kernel for 8 Trainium2 NeuronCores.

Sharding: core c handles batch b = c//2 and head-group hg = c%2 (8 of 16
heads). Each core computes qkv projection for its heads, head-parallel
causal attention, and a partial output projection; the host sums the two
head-group partials per batch and adds the bias terms.

The Q/K projections run in fp8 (e4m3) DoubleRow perf mode: x and the
q/k weight blocks are quantized to fp8 (weights pre-scaled x32 so fp8
stays in normal range) and the contraction C=1024 is packed as 4
k-tiles of 256 (2 sub-tiles per partition), halving the projection
matmul count. The exp activation folds the compensating 2^-13 scale
(= 0.125 softmax scale / 32^2). Softmax averaging absorbs the ~4% fp8
noise (~1.1e-2 rel total); V/P/otn/wp stay bf16 - fp8 noise on any
tensor feeding the 512-long output contraction alone costs ~2e-2.

Schedule: all projection work (QK chains, V chains, output projection)
is decomposed into small units fed one-per-iteration into the attention
loop so the PE (the bottleneck engine, ~82% busy) never runs dry.
Softmax normalization for chunk qc is emitted after chunk qc+1's first
scores so neither PE nor ACT stalls at boundaries.

On-chip layouts (per core):
  xT  [C, T] as 4 column-chunks [128,512] x 8 k-tiles (bf16, for V)
  x8  fp8 copy of x, paired k-tiles [128, 2(cg), 512] x 4 (for QK-DR)
  Q^T/K^T per pair    [128 = headA dims | headB dims, T] (bf16)
  V   per token tile  [128, 8*65] - per-head stripes [V_h | ones] so the
                      P@V matmul's ones column accumulates softmax sums
  S^T per (kt,qc)     [keys=128, queries<=512] in PSUM; exp on ScalarE
  O^T per (pair,qc)   [128,1024] PSUM: head A cols 0:512, head B 512:1024;
                      row 64 of each half = softmax sums
  softmax denominators broadcast across partitions via K=1 f32r matmul
"""
import sys
sys.path.insert(0, '/opt/trn_rl_repo')
from collections import deque
import numpy as np
import ml_dtypes

from concourse import bacc, mybir
import concourse.tile as tile
from concourse.bass_utils import run_bass_kernel_spmd

B, T, C, H = 4, 2048, 1024, 16
D = C // H           # 64
HPC = H // 2         # 8 heads per core
NPAIR = HPC // 2     # 4
N_CORES = 8
KC = C // 128        # 8 contraction tiles for projections
BF = mybir.dt.bfloat16
F32 = mybir.dt.float32
F32R = mybir.dt.float32r
I32 = mybir.dt.int32
FP8 = mybir.dt.float8e4
DR = mybir.MatmulPerfMode.DoubleRow
FP8_NP = ml_dtypes.float8_e4m3
EXPSCALE = float(2.0 ** -13)
W_SCALE = 32.0
SCH_A = float((1 << 23) / np.log(2))
SCH_B = float(127 * (1 << 23) - int(0.0430 * (1 << 23)))
USE_SCHRAUDOLPH = False
S_PERF_MODE = None
QK_DT = mybir.dt.bfloat16
BF_NP = ml_dtypes.bfloat16

_CACHE = {}


def build(t=T):
    QC = t // 512        # query chunks
    TT = t // 128        # token/key tiles
    nc = bacc.Bacc("TRN2", target_bir_lowering=False, debug=False,
                   num_devices=N_CORES)
    xTd = nc.dram_tensor("xT", [QC, C, 512], BF, kind="ExternalInput").ap()
    x8d = nc.dram_tensor("x8", [QC, KC // 2, 128, 1024], FP8,
                         kind="ExternalInput").ap()
    wqk = nc.dram_tensor("wqk", [NPAIR, 2, KC // 2, 128, 256], FP8,
                         kind="ExternalInput").ap()
    wv = nc.dram_tensor("wv", [C, HPC * D], BF, kind="ExternalInput").ap()
    wp = nc.dram_tensor("wp", [HPC * D, C], BF, kind="ExternalInput").ap()
    bq = nc.dram_tensor("bq", [128, NPAIR], F32, kind="ExternalInput").ap()
    masks = nc.dram_tensor("masks", [128, 128], BF, kind="ExternalInput").ap()
    y = nc.dram_tensor("y", [t, C], BF, kind="ExternalOutput").ap()

    with tile.TileContext(nc) as tc:
        with tc.tile_pool(name="const", bufs=1) as cpool, \
             tc.tile_pool(name="work", bufs=1) as wpool, \
             tc.tile_pool(name="psS", bufs=2, space="PSUM") as psS, \
             tc.tile_pool(name="psO", bufs=1, space="PSUM") as psO, \
             tc.tile_pool(name="psC", bufs=2, space="PSUM") as psC, \
             tc.tile_pool(name="att", bufs=12) as att_pool, \
             tc.tile_pool(name="nrm", bufs=2) as nrm_pool, \
             tc.tile_pool(name="bits", bufs=2) as bits_pool, \
             tc.tile_pool(name="yo", bufs=3) as y_pool:

            # ---- tiny constants first (vector-engine DMA queue)
            mask_sb = cpool.tile([128, 128], BF, tag="mask")
            nc.sync.dma_start(mask_sb[:], masks)
            bq_sb = cpool.tile([128, NPAIR], F32, tag="bq")
            nc.sync.dma_start(bq_sb[:], bq)
            ones32 = cpool.tile([128, 64], F32, tag="ones32")
            nc.vector.memset(ones32[:], 1.0)
            ones_fr = cpool.tile([128, 64], F32R, tag="ones_fr")
            nc.vector.tensor_copy(ones_fr[:], ones32[:])

            # ---- weights / activations DMA, critical-path order:
            # pair0 wqk block + xT chunk 0 + wv, then the rest, wp last
            wqk_sb = {}
            xT_sb = {}
            x8_sb = {}
            wv_sb = []
            tl = cpool.tile([128, 2, KC // 2, 256], FP8, tag="wqk0",
                            name="wqk0")
            nc.scalar.dma_start(
                tl[:], wqk[0].rearrange("w k p m -> p w k m"))
            wqk_sb[0] = tl
            for k in range(KC // 2):
                tl = cpool.tile([128, 2, 512], FP8, tag=f"x8{k}_0",
                                name=f"x8{k}_0")
                eng = nc.sync if k % 2 == 0 else nc.scalar
                eng.dma_start(
                    tl[:], x8d[0, k].rearrange("p (c t) -> p c t", c=2))
                x8_sb[(k, 0)] = tl
            for k in range(KC):
                tl = cpool.tile([128, 512], BF, tag=f"xT{k}_0", name=f"xT{k}_0")
                eng = nc.sync if k % 2 == 0 else nc.gpsimd
                eng.dma_start(tl[:], xTd[0, k * 128:(k + 1) * 128, :])
                xT_sb[(k, 0)] = tl
            # x8 chunk 1 ahead of wv: qk chunk-1 units gate the (0,1)
            # attention chunk earlier than the wv-consuming PV does
            if QC > 1:
                for k in range(KC // 2):
                    tl = cpool.tile([128, 2, 512], FP8, tag=f"x8{k}_1",
                                    name=f"x8{k}_1")
                    eng = nc.scalar if k % 2 == 0 else nc.sync
                    eng.dma_start(
                        tl[:], x8d[1, k].rearrange("p (c t) -> p c t", c=2))
                    x8_sb[(k, 1)] = tl
            for k in range(KC):
                tl = cpool.tile([128, HPC * D], BF, tag=f"wv{k}", name=f"wv{k}")
                nc.scalar.dma_start(tl[:], wv[k * 128:(k + 1) * 128, :])
                wv_sb.append(tl)
            for p in range(1, NPAIR):
                tl = cpool.tile([128, 2, KC // 2, 256], FP8, tag=f"wqk{p}",
                                name=f"wqk{p}")
                nc.gpsimd.dma_start(
                    tl[:], wqk[p].rearrange("w k p m -> p w k m"))
                wqk_sb[p] = tl
            for tch in range(1, QC):
                for k in range(KC // 2):
                    if (k, tch) in x8_sb:
                        continue
                    tl = cpool.tile([128, 2, 512], FP8, tag=f"x8{k}_{tch}",
                                    name=f"x8{k}_{tch}")
                    eng = nc.scalar if k % 2 == 0 else nc.sync
                    eng.dma_start(
                        tl[:], x8d[tch, k].rearrange("p (c t) -> p c t", c=2))
                    x8_sb[(k, tch)] = tl
                for k in range(KC):
                    tl = cpool.tile([128, 512], BF, tag=f"xT{k}_{tch}",
                                    name=f"xT{k}_{tch}")
                    eng = nc.sync if k % 2 == 0 else nc.gpsimd
                    eng.dma_start(tl[:], xTd[tch, k * 128:(k + 1) * 128, :])
                    xT_sb[(k, tch)] = tl
            wp_sb = []
            for p in range(NPAIR):
                tl = cpool.tile([128, C], BF, tag=f"wp{p}", name=f"wp{p}")
                nc.scalar.dma_start(tl[:], wp[p * 128:(p + 1) * 128, :])
                wp_sb.append(tl)

            # persistent intermediates
            qt_sb = [wpool.tile([128, t], QK_DT, tag=f"qt{p}", name=f"qt{p}")
                     for p in range(NPAIR)]
            kt_sb = [wpool.tile([128, t], QK_DT, tag=f"kt{p}", name=f"kt{p}")
                     for p in range(NPAIR)]
            v_sb = [wpool.tile([128, HPC * 65], BF, tag=f"v{i}", name=f"v{i}")
                    for i in range(TT)]
            otn_sb = [wpool.tile([128, t], BF, tag=f"otn{p}", name=f"otn{p}")
                      for p in range(NPAIR)]

            # ---- chain units (projection work fed into the attention loop)
            def emit_qk_unit(p, which, tch):
                # which: 0=q, 1=k ; writes qt/kt chunk tch for pair p.
                # fp8 DoubleRow: 4 k-tiles of 256 contraction.
                ps = psC.tile([128, 512], F32, tag="c",
                              name=f"qk{p}_{which}_{tch}")
                for k in range(KC // 2):
                    nc.tensor.matmul(
                        ps[:],
                        wqk_sb[p][:, which, k]
                        .rearrange("p (c m) -> p c m", c=2),
                        x8_sb[(k, tch)][:],
                        start=(k == 0), stop=(k == KC // 2 - 1), perf_mode=DR)
                dst = (qt_sb if which == 0 else kt_sb)[p]
                if which == 0:
                    nc.vector.tensor_scalar_add(
                        dst[:, tch * 512:(tch + 1) * 512], ps[:],
                        bq_sb[:, p:p + 1])
                else:
                    nc.vector.tensor_copy(
                        dst[:, tch * 512:(tch + 1) * 512], ps[:])

            def emit_v_unit(i):
                ps = psC.tile([128, 512], F32, tag="c", name=f"v{i}")
                tch, j = divmod(i, 4)
                for k in range(KC):
                    nc.tensor.matmul(
                        ps[:, 0:HPC * D],
                        xT_sb[(k, tch)][:, j * 128:(j + 1) * 128],
                        wv_sb[k][:], start=(k == 0), stop=(k == KC - 1))
                vt = v_sb[i][:].rearrange("p (h e) -> p h e", e=65)
                nc.vector.memset(vt[:, :, 64:65], 1.0)
                nc.vector.tensor_copy(
                    vt[:, :, 0:64],
                    ps[:, 0:HPC * D].rearrange("p (h d) -> p h d", d=64))

            def emit_yproj_unit(i, ch):
                ps = psC.tile([128, 512], F32, tag="c", name=f"y{i}_{ch}")
                for p in range(NPAIR):
                    nc.tensor.matmul(
                        ps[:], otn_sb[p][:, i * 128:(i + 1) * 128],
                        wp_sb[p][:, ch * 512:(ch + 1) * 512],
                        start=(p == 0), stop=(p == NPAIR - 1))
                ysb = y_pool.tile([128, 512], BF, tag="ysb", name=f"ysb{i}_{ch}")
                nc.vector.tensor_copy(ysb[:], ps[:])
                nc.sync.dma_start(
                    y[i * 128:(i + 1) * 128, ch * 512:(ch + 1) * 512], ysb[:])

            feeder = []   # entries: (deadline_seq_idx, order, callable)
            _fcnt = [0]

            def fpush(dl, fn):
                # keep sorted by (deadline, insertion order) so hard
                # deadlines are never stuck behind unbounded (yproj) work
                import bisect
                _fcnt[0] += 1
                bisect.insort(feeder, (dl, _fcnt[0], fn))

            def feed(n):
                for _ in range(n):
                    if not feeder:
                        return
                    feeder.pop(0)[2]()

            def force_feed(idx_limit):
                # emit every unit that must precede seq[idx_limit]
                while feeder and feeder[0][0] <= idx_limit:
                    feeder.pop(0)[2]()

            # ---- prologue: first chains so PE starts as soon as DMA lands
            emit_qk_unit(0, 0, 0)
            emit_qk_unit(0, 1, 0)
            for i in range(min(4, TT)):
                emit_v_unit(i)

            def emit_ot(p, pend_item, ps_o, ktmax):
                kt, lo, pt = pend_item
                for hi in (0, 1):
                    h = 2 * p + hi
                    nc.tensor.matmul(
                        ps_o[0:65, hi * 512 + lo:hi * 512 + 512],
                        v_sb[kt][:, 65 * h:65 * h + 65],
                        pt[:, hi * 512 + lo:(hi + 1) * 512],
                        start=(kt == 0), stop=(kt == ktmax))

            def emit_s_exp(p, qc, kt):
                delta = kt - 4 * qc
                lo = 128 * delta if delta > 0 else 0
                ps_s = psS.tile([128, 1024], F32, tag="S",
                                name=f"s{p}_{qc}_{kt}")
                for hi, (p0, p1) in enumerate(((0, 64), (64, 128))):
                    nc.tensor.matmul(
                        ps_s[:, hi * 512 + lo:(hi + 1) * 512],
                        kt_sb[p][p0:p1, kt * 128:(kt + 1) * 128],
                        qt_sb[p][p0:p1, qc * 512 + lo:(qc + 1) * 512],
                        start=True, stop=True, perf_mode=S_PERF_MODE)
                pt = att_pool.tile([128, 1024], BF, tag="pt",
                                   name=f"pt{p}_{qc}_{kt}")
                if USE_SCHRAUDOLPH and p >= 1 and qc >= 2 and delta < 0 and kt % 4 == 3:
                    # Schraudolph exp on DVE to relieve ACT
                    bits = bits_pool.tile([128, 1024], I32, tag="bits",
                                          name=f"bits{p}_{qc}_{kt}")
                    nc.vector.tensor_scalar(
                        bits[:], ps_s[:], SCH_A, SCH_B,
                        op0=mybir.AluOpType.mult,
                        op1=mybir.AluOpType.add)
                    nc.vector.tensor_copy(pt[:], bits[:].bitcast(F32))
                else:
                    nc.scalar.activation(
                        pt[:].rearrange("p (u c) -> p u c", u=2)[:, :, lo:512],
                        ps_s[:].rearrange("p (u c) -> p u c", u=2)[:, :, lo:512],
                        mybir.ActivationFunctionType.Exp, scale=EXPSCALE)
                if delta >= 0:
                    # only cols [lo, lo+128) straddle the causal boundary
                    for hi in range(2):
                        sl = pt[:, hi * 512 + lo:hi * 512 + lo + 128]
                        nc.vector.tensor_mul(sl, sl, mask_sb[:])
                return (kt, lo, pt)

            def normalize(p, qc, ps_o, otnB):
                qs = slice(qc * 512, (qc + 1) * 512)
                sums_fr = nrm_pool.tile([128, 1024], F32R, tag="sums",
                                        name=f"sums{p}_{qc}")
                nc.vector.tensor_copy(sums_fr[64:65, 0:1024],
                                      ps_o[64:65, 0:1024])
                bc_sb = nrm_pool.tile([64, 1024], F32, tag="bc",
                                      name=f"bc{p}_{qc}")
                for hi in (0, 1):
                    ps_bc = psC.tile([128, 512], F32, tag="c",
                                     name=f"bc{p}_{qc}_{hi}")
                    nc.tensor.matmul(
                        ps_bc[0:64, :], ones_fr[64:65, 0:64],
                        sums_fr[64:65, hi * 512:(hi + 1) * 512],
                        start=True, stop=True)
                    nc.vector.reciprocal_approx_fast(
                        bc_sb[0:64, hi * 512:(hi + 1) * 512],
                        ps_bc[0:64, :])
                nc.vector.tensor_mul(
                    otn_sb[p][0:64, qs], ps_o[0:64, 0:512],
                    bc_sb[0:64, 0:512])
                nc.vector.tensor_mul(
                    otnB[0:64, qs], ps_o[0:64, 512:1024],
                    bc_sb[0:64, 512:1024])
                nc.sync.dma_start(otn_sb[p][64:128, qs], otnB[0:64, qs])
                if p == NPAIR - 1:
                    for i in range(4 * qc, min(4 * qc + 4, TT)):
                        for ch in range(C // 512):
                            fpush(10 ** 9,
                                  lambda i=i, ch=ch: emit_yproj_unit(i, ch))

            seq = [(p, qc) for p in range(NPAIR) for qc in range(QC)]
            otnB_t = [nrm_pool.tile([64, t], BF, tag="otnB", name=f"otnB{p}")
                      for p in range(NPAIR)]
            # preseed the very first chunk's first two key tiles
            pend_next = [emit_s_exp(0, 0, 0), emit_s_exp(0, 0, 1)]

            for idx, (p, qc) in enumerate(seq):
                fn = 2 if p in (0, NPAIR - 1) else 1
                if qc + 1 < QC and (p == 0 or qc > 0):
                    # chunk qc+1 for this pair (chunk 1 of pairs >0 was
                    # already queued by the previous pair)
                    fpush(idx + 1, lambda p=p, tc_=qc + 1: emit_qk_unit(p, 0, tc_))
                    fpush(idx + 1, lambda p=p, tc_=qc + 1: emit_qk_unit(p, 1, tc_))
                if p == 0 and qc + 1 < QC:
                    for i in range(4 * (qc + 1), min(4 * (qc + 2), TT)):
                        fpush(idx + 1, lambda i=i: emit_v_unit(i))
                if p + 1 < NPAIR and qc == max(0, QC - 2):
                    nidx = (p + 1) * QC
                    fpush(nidx, lambda p=p: emit_qk_unit(p + 1, 0, 0))
                    fpush(nidx, lambda p=p: emit_qk_unit(p + 1, 1, 0))
                    if QC > 1:
                        fpush(nidx + 1, lambda p=p: emit_qk_unit(p + 1, 0, 1))
                        fpush(nidx + 1, lambda p=p: emit_qk_unit(p + 1, 1, 1))

                ktmax = 4 * qc + 3
                ps_o = psO.tile([128, 1024], F32, tag="O", name=f"o{p}_{qc}")
                pend = deque(pend_next)
                pend_next = []
                for ktp in range(2, ktmax + 1, 2):
                    pend.append(emit_s_exp(p, qc, ktp))
                    pend.append(emit_s_exp(p, qc, ktp + 1))
                    while len(pend) > 4:
                        emit_ot(p, pend.popleft(), ps_o, ktmax)
                    if ktp < ktmax - 1:
                        feed(fn)
                # pre-emit the next chunk's first scores/exp so ACT keeps
                # running across the boundary while we drain and normalize
                if idx + 1 < len(seq):
                    np_, nqc = seq[idx + 1]
                    force_feed(idx + 1)
                    pend_next = [emit_s_exp(np_, nqc, 0), emit_s_exp(np_, nqc, 1)]
                while pend:
                    emit_ot(p, pend.popleft(), ps_o, ktmax)
                normalize(p, qc, ps_o, otnB_t[p])
                feed(fn)
            feed(len(feeder))

    nc.compile()
    return nc


def get_nc(t=T):
    if t not in _CACHE:
        _CACHE[t] = build(t)
    return _CACHE[t]


def make_masks():
    r = np.arange(128)[:, None]
    c = np.arange(128)[None, :]
    return (r <= c).astype(BF_NP)


def prep_inputs(x, W_attn, b_attn, W_proj, t=T):
    """Per-core input maps. Core c: batch c//2, head group c%2."""
    masks = make_masks()
    qs, ks, vs = W_attn[:, :C], W_attn[:, C:2 * C], W_attn[:, 2 * C:]
    bqs = b_attn[:C]
    QCt = t // 512
    in_maps = []
    for core in range(N_CORES):
        b, hg = core // 2, core % 2
        xb = x[b, :t].astype(np.float32)
        xTb = np.ascontiguousarray(
            xb.T.reshape(C, QCt, 512).transpose(1, 0, 2)).astype(BF_NP)
        # x8: [QC, KC//2, 128, cg*512+t] fp8
        x8 = np.ascontiguousarray(
            xb.astype(FP8_NP).astype(np.float32).T
            .reshape(KC // 2, 2, 128, QCt, 512).transpose(3, 0, 2, 1, 4)
            .reshape(QCt, KC // 2, 128, 1024)).astype(FP8_NP)
        # wqk: [NPAIR, which, KC//2, p, cg*128 + m] fp8, x32
        wqk_arr = np.zeros((NPAIR, 2, KC // 2, 128, 2, 128), dtype=FP8_NP)
        bqcols = []
        for p in range(NPAIR):
            ha = hg * HPC + 2 * p
            for which, wsrc in enumerate((qs, ks)):
                blk = wsrc[:, ha * D:(ha + 2) * D] * W_SCALE   # [C, 128]
                wqk_arr[p, which] = blk.reshape(
                    KC // 2, 2, 128, 128).transpose(0, 2, 1, 3).astype(FP8_NP)
            bqcols.append(bqs[ha * D:(ha + 2) * D] * W_SCALE)
        wqk = np.ascontiguousarray(
            wqk_arr.reshape(NPAIR, 2, KC // 2, 128, 256))
        wv = vs[:, hg * HPC * D:(hg + 1) * HPC * D].astype(BF_NP)
        wp = W_proj[hg * HPC * D:(hg + 1) * HPC * D, :].astype(BF_NP)
        bq = np.stack(bqcols, axis=1).astype(np.float32)
        in_maps.append({"xT": xTb, "x8": x8, "wqk": wqk, "wv": wv, "wp": wp,
                        "bq": bq, "masks": masks})
    return in_maps


def kernel(x, W_attn, b_attn, W_proj, b_proj):
    x = np.asarray(x, dtype=np.float32)
    W_attn = np.asarray(W_attn, dtype=np.float32)
    b_attn = np.asarray(b_attn, dtype=np.float32)
    W_proj = np.asarray(W_proj, dtype=np.float32)
    b_proj = np.asarray(b_proj, dtype=np.float32)

    nc = get_nc(T)
    in_maps = prep_inputs(x, W_attn, b_attn, W_proj, T)
    res = run_bass_kernel_spmd(nc, in_maps, list(range(N_CORES)))
    # host reduction: sum the two head-group partials; bias terms
    bias = b_attn[2 * C:] @ W_proj + b_proj  # v-bias passes through linearly
    y = np.empty((B, T, C), dtype=np.float32)
    for b in range(B):
        y[b] = (res.results[2 * b]["y"].astype(np.float32)
                + res.results[2 * b + 1]["y"].astype(np.float32) + bias)
    return y



# revision 53
# speedup vs baseline: 1.0015x; 1.0015x over previous
"""Causal self-attention kernel for 8 Trainium2 NeuronCores.

Sharding: core c handles batch b = c//2 and head-group hg = c%2 (8 of 16
heads). Each core computes qkv projection for its heads, head-parallel
causal attention, and a partial output projection; the host sums the two
head-group partials per batch and adds the bias terms.

The Q/K projections run in fp8 (e4m3) DoubleRow perf mode: x and the
q/k weight blocks are quantized to fp8 (weights pre-scaled x32 so fp8
stays in normal range) and the contraction C=1024 is packed as 4
k-tiles of 256 (2 sub-tiles per partition), halving the projection
matmul count. The exp activation folds the compensating 2^-13 scale
(= 0.125 softmax scale / 32^2). Softmax averaging absorbs the ~4% fp8
noise (~1.1e-2 rel total); V/P/otn/wp stay bf16 - fp8 noise on any
tensor feeding the 512-long output contraction alone costs ~2e-2.

Schedule: all projection work (QK chains, V chains, output projection)
is decomposed into small units fed one-per-iteration into the attention
loop so the PE (the bottleneck engine, ~82% busy) never runs dry.
Softmax normalization for chunk qc is emitted after chunk qc+1's first
scores so neither PE nor ACT stalls at boundaries.

On-chip layouts (per core):
  xT  [C, T] as 4 column-chunks [128,512] x 8 k-tiles (bf16, for V)
  x8  fp8 copy of x, paired k-tiles [128, 2(cg), 512] x 4 (for QK-DR)
  Q^T/K^T per pair    [128 = headA dims | headB dims, T] (bf16)
  V   per token tile  [128, 8*65] - per-head stripes [V_h | ones] so the
                      P@V matmul's ones column accumulates softmax sums
  S^T per (kt,qc)     [keys=128, queries<=512] in PSUM; exp on ScalarE
  O^T per (pair,qc)   [128,1024] PSUM: head A cols 0:512, head B 512:1024;
                      row 64 of each half = softmax sums
  softmax denominators broadcast across partitions via K=1 f32r matmul
"""
import sys
sys.path.insert(0, '/opt/trn_rl_repo')
from collections import deque
import numpy as np
import ml_dtypes

from concourse import bacc, mybir
import concourse.tile as tile
from concourse.bass_utils import run_bass_kernel_spmd

B, T, C, H = 4, 2048, 1024, 16
D = C // H           # 64
HPC = H // 2         # 8 heads per core
NPAIR = HPC // 2     # 4
N_CORES = 8
KC = C // 128        # 8 contraction tiles for projections
BF = mybir.dt.bfloat16
F32 = mybir.dt.float32
F32R = mybir.dt.float32r
I32 = mybir.dt.int32
FP8 = mybir.dt.float8e4
DR = mybir.MatmulPerfMode.DoubleRow
FP8_NP = ml_dtypes.float8_e4m3
EXPSCALE = float(2.0 ** -13)
W_SCALE = 32.0
SCH_A = float((1 << 23) / np.log(2))
SCH_B = float(127 * (1 << 23) - int(0.0430 * (1 << 23)))
USE_SCHRAUDOLPH = False
S_PERF_MODE = None
QK_DT = mybir.dt.bfloat16
BF_NP = ml_dtypes.bfloat16

_CACHE = {}


def build(t=T):
    QC = t // 512        # query chunks
    TT = t // 128        # token/key tiles
    nc = bacc.Bacc("TRN2", target_bir_lowering=False, debug=False,
                   num_devices=N_CORES)
    xTd = nc.dram_tensor("xT", [QC, C, 512], BF, kind="ExternalInput").ap()
    x8d = nc.dram_tensor("x8", [QC, KC // 2, 128, 1024], FP8,
                         kind="ExternalInput").ap()
    wqk = nc.dram_tensor("wqk", [NPAIR, 2, KC // 2, 128, 256], FP8,
                         kind="ExternalInput").ap()
    wv = nc.dram_tensor("wv", [C, HPC * D], BF, kind="ExternalInput").ap()
    wp = nc.dram_tensor("wp", [HPC * D, C], BF, kind="ExternalInput").ap()
    bq = nc.dram_tensor("bq", [128, NPAIR], F32, kind="ExternalInput").ap()
    masks = nc.dram_tensor("masks", [128, 128], BF, kind="ExternalInput").ap()
    y = nc.dram_tensor("y", [t, C], BF, kind="ExternalOutput").ap()

    with tile.TileContext(nc) as tc:
        with tc.tile_pool(name="const", bufs=1) as cpool, \
             tc.tile_pool(name="work", bufs=1) as wpool, \
             tc.tile_pool(name="psS", bufs=2, space="PSUM") as psS, \
             tc.tile_pool(name="psO", bufs=1, space="PSUM") as psO, \
             tc.tile_pool(name="psC", bufs=2, space="PSUM") as psC, \
             tc.tile_pool(name="att", bufs=12) as att_pool, \
             tc.tile_pool(name="nrm", bufs=2) as nrm_pool, \
             tc.tile_pool(name="bits", bufs=2) as bits_pool, \
             tc.tile_pool(name="yo", bufs=3) as y_pool:

            # ---- tiny constants first (vector-engine DMA queue)
            mask_sb = cpool.tile([128, 128], BF, tag="mask")
            nc.sync.dma_start(mask_sb[:], masks)
            bq_sb = cpool.tile([128, NPAIR], F32, tag="bq")
            nc.sync.dma_start(bq_sb[:], bq)
            ones32 = cpool.tile([128, 64], F32, tag="ones32")
            nc.vector.memset(ones32[:], 1.0)
            ones_fr = cpool.tile([128, 64], F32R, tag="ones_fr")
            nc.vector.tensor_copy(ones_fr[:], ones32[:])

            # ---- weights / activations DMA, critical-path order:
            # pair0 wqk block + xT chunk 0 + wv, then the rest, wp last
            wqk_sb = {}
            xT_sb = {}
            x8_sb = {}
            wv_sb = []
            tl = cpool.tile([128, 2, KC // 2, 256], FP8, tag="wqk0",
                            name="wqk0")
            nc.scalar.dma_start(
                tl[:], wqk[0].rearrange("w k p m -> p w k m"))
            wqk_sb[0] = tl
            for k in range(KC // 2):
                tl = cpool.tile([128, 2, 512], FP8, tag=f"x8{k}_0",
                                name=f"x8{k}_0")
                eng = nc.sync if k % 2 == 0 else nc.scalar
                eng.dma_start(
                    tl[:], x8d[0, k].rearrange("p (c t) -> p c t", c=2))
                x8_sb[(k, 0)] = tl
            for k in range(KC):
                tl = cpool.tile([128, 512], BF, tag=f"xT{k}_0", name=f"xT{k}_0")
                eng = nc.sync if k % 2 == 0 else nc.gpsimd
                eng.dma_start(tl[:], xTd[0, k * 128:(k + 1) * 128, :])
                xT_sb[(k, 0)] = tl
            # x8 chunk 1 ahead of wv: qk chunk-1 units gate the (0,1)
            # attention chunk earlier than the wv-consuming PV does
            if QC > 1:
                for k in range(KC // 2):
                    tl = cpool.tile([128, 2, 512], FP8, tag=f"x8{k}_1",
                                    name=f"x8{k}_1")
                    eng = nc.scalar if k % 2 == 0 else nc.sync
                    eng.dma_start(
                        tl[:], x8d[1, k].rearrange("p (c t) -> p c t", c=2))
                    x8_sb[(k, 1)] = tl
            for k in range(KC):
                tl = cpool.tile([128, HPC * D], BF, tag=f"wv{k}", name=f"wv{k}")
                nc.scalar.dma_start(tl[:], wv[k * 128:(k + 1) * 128, :])
                wv_sb.append(tl)
            for p in range(1, NPAIR):
                tl = cpool.tile([128, 2, KC // 2, 256], FP8, tag=f"wqk{p}",
                                name=f"wqk{p}")
                nc.gpsimd.dma_start(
                    tl[:], wqk[p].rearrange("w k p m -> p w k m"))
                wqk_sb[p] = tl
            for tch in range(1, QC):
                for k in range(KC // 2):
                    if (k, tch) in x8_sb:
                        continue
                    tl = cpool.tile([128, 2, 512], FP8, tag=f"x8{k}_{tch}",
                                    name=f"x8{k}_{tch}")
                    eng = nc.scalar if k % 2 == 0 else nc.sync
                    eng.dma_start(
                        tl[:], x8d[tch, k].rearrange("p (c t) -> p c t", c=2))
                    x8_sb[(k, tch)] = tl
                for k in range(KC):
                    tl = cpool.tile([128, 512], BF, tag=f"xT{k}_{tch}",
                                    name=f"xT{k}_{tch}")
                    eng = nc.sync if k % 2 == 0 else nc.gpsimd
                    eng.dma_start(tl[:], xTd[tch, k * 128:(k + 1) * 128, :])
                    xT_sb[(k, tch)] = tl
            wp_sb = []
            for p in range(NPAIR):
                tl = cpool.tile([128, C], BF, tag=f"wp{p}", name=f"wp{p}")
                nc.scalar.dma_start(tl[:], wp[p * 128:(p + 1) * 128, :])
                wp_sb.append(tl)

            # persistent intermediates
            qt_sb = [wpool.tile([128, t], QK_DT, tag=f"qt{p}", name=f"qt{p}")
                     for p in range(NPAIR)]
            kt_sb = [wpool.tile([128, t], QK_DT, tag=f"kt{p}", name=f"kt{p}")
                     for p in range(NPAIR)]
            v_sb = [wpool.tile([128, HPC * 65], BF, tag=f"v{i}", name=f"v{i}")
                    for i in range(TT)]
            otn_sb = [wpool.tile([128, t], BF, tag=f"otn{p}", name=f"otn{p}")
                      for p in range(NPAIR)]

            # ---- chain units (projection work fed into the attention loop)
            def emit_qk_unit(p, which, tch):
                # which: 0=q, 1=k ; writes qt/kt chunk tch for pair p.
                # fp8 DoubleRow: 4 k-tiles of 256 contraction.
                ps = psC.tile([128, 512], F32, tag="c",
                              name=f"qk{p}_{which}_{tch}")
                for k in range(KC // 2):
                    nc.tensor.matmul(
                        ps[:],
                        wqk_sb[p][:, which, k]
                        .rearrange("p (c m) -> p c m", c=2),
                        x8_sb[(k, tch)][:],
                        start=(k == 0), stop=(k == KC // 2 - 1), perf_mode=DR)
                dst = (qt_sb if which == 0 else kt_sb)[p]
                if which == 0:
                    nc.vector.tensor_scalar_add(
                        dst[:, tch * 512:(tch + 1) * 512], ps[:],
                        bq_sb[:, p:p + 1])
                else:
                    nc.vector.tensor_copy(
                        dst[:, tch * 512:(tch + 1) * 512], ps[:])

            def emit_v_unit(i):
                ps = psC.tile([128, 512], F32, tag="c", name=f"v{i}")
                tch, j = divmod(i, 4)
                for k in range(KC):
                    nc.tensor.matmul(
                        ps[:, 0:HPC * D],
                        xT_sb[(k, tch)][:, j * 128:(j + 1) * 128],
                        wv_sb[k][:], start=(k == 0), stop=(k == KC - 1))
                vt = v_sb[i][:].rearrange("p (h e) -> p h e", e=65)
                nc.vector.memset(vt[:, :, 64:65], 1.0)
                nc.vector.tensor_copy(
                    vt[:, :, 0:64],
                    ps[:, 0:HPC * D].rearrange("p (h d) -> p h d", d=64))

            def emit_yproj_unit(i, ch):
                ps = psC.tile([128, 512], F32, tag="c", name=f"y{i}_{ch}")
                for p in range(NPAIR):
                    nc.tensor.matmul(
                        ps[:], otn_sb[p][:, i * 128:(i + 1) * 128],
                        wp_sb[p][:, ch * 512:(ch + 1) * 512],
                        start=(p == 0), stop=(p == NPAIR - 1))
                ysb = y_pool.tile([128, 512], BF, tag="ysb", name=f"ysb{i}_{ch}")
                nc.vector.tensor_copy(ysb[:], ps[:])
                nc.sync.dma_start(
                    y[i * 128:(i + 1) * 128, ch * 512:(ch + 1) * 512], ysb[:])

            feeder = []   # entries: (deadline_seq_idx, order, callable)
            _fcnt = [0]

            def fpush(dl, fn):
                # keep sorted by (deadline, insertion order) so hard
                # deadlines are never stuck behind unbounded (yproj) work
                import bisect
                _fcnt[0] += 1
                bisect.insort(feeder, (dl, _fcnt[0], fn))

            def feed(n):
                for _ in range(n):
                    if not feeder:
                        return
                    feeder.pop(0)[2]()

            def force_feed(idx_limit):
                # emit every unit that must precede seq[idx_limit]
                while feeder and feeder[0][0] <= idx_limit:
                    feeder.pop(0)[2]()

            # ---- prologue: first chains so PE starts as soon as DMA lands
            emit_qk_unit(0, 0, 0)
            emit_qk_unit(0, 1, 0)
            for i in range(min(4, TT)):
                emit_v_unit(i)

            def emit_ot(p, pend_item, ps_o, ktmax):
                kt, lo, pt = pend_item
                for hi in (0, 1):
                    h = 2 * p + hi
                    nc.tensor.matmul(
                        ps_o[0:65, hi * 512 + lo:hi * 512 + 512],
                        v_sb[kt][:, 65 * h:65 * h + 65],
                        pt[:, hi * 512 + lo:(hi + 1) * 512],
                        start=(kt == 0), stop=(kt == ktmax))

            def emit_s_exp(p, qc, kt):
                delta = kt - 4 * qc
                lo = 128 * delta if delta > 0 else 0
                ps_s = psS.tile([128, 1024], F32, tag="S",
                                name=f"s{p}_{qc}_{kt}")
                for hi, (p0, p1) in enumerate(((0, 64), (64, 128))):
                    nc.tensor.matmul(
                        ps_s[:, hi * 512 + lo:(hi + 1) * 512],
                        kt_sb[p][p0:p1, kt * 128:(kt + 1) * 128],
                        qt_sb[p][p0:p1, qc * 512 + lo:(qc + 1) * 512],
                        start=True, stop=True, perf_mode=S_PERF_MODE)
                pt = att_pool.tile([128, 1024], BF, tag="pt",
                                   name=f"pt{p}_{qc}_{kt}")
                if USE_SCHRAUDOLPH and p >= 1 and qc >= 2 and delta < 0 and kt % 4 == 3:
                    # Schraudolph exp on DVE to relieve ACT
                    bits = bits_pool.tile([128, 1024], I32, tag="bits",
                                          name=f"bits{p}_{qc}_{kt}")
                    nc.vector.tensor_scalar(
                        bits[:], ps_s[:], SCH_A, SCH_B,
                        op0=mybir.AluOpType.mult,
                        op1=mybir.AluOpType.add)
                    nc.vector.tensor_copy(pt[:], bits[:].bitcast(F32))
                else:
                    nc.scalar.activation(
                        pt[:].rearrange("p (u c) -> p u c", u=2)[:, :, lo:512],
                        ps_s[:].rearrange("p (u c) -> p u c", u=2)[:, :, lo:512],
                        mybir.ActivationFunctionType.Exp, scale=EXPSCALE)
                if delta >= 0:
                    # only cols [lo, lo+128) straddle the causal boundary
                    for hi in range(2):
                        sl = pt[:, hi * 512 + lo:hi * 512 + lo + 128]
                        nc.vector.tensor_mul(sl, sl, mask_sb[:])
                return (kt, lo, pt)

            def normalize(p, qc, ps_o, otnB):
                qs = slice(qc * 512, (qc + 1) * 512)
                sums_fr = nrm_pool.tile([128, 1024], F32R, tag="sums",
                                        name=f"sums{p}_{qc}")
                nc.vector.tensor_copy(sums_fr[64:65, 0:1024],
                                      ps_o[64:65, 0:1024])
                bc_sb = nrm_pool.tile([64, 1024], F32, tag="bc",
                                      name=f"bc{p}_{qc}")
                for hi in (0, 1):
                    ps_bc = psC.tile([128, 512], F32, tag="c",
                                     name=f"bc{p}_{qc}_{hi}")
                    nc.tensor.matmul(
                        ps_bc[0:64, :], ones_fr[64:65, 0:64],
                        sums_fr[64:65, hi * 512:(hi + 1) * 512],
                        start=True, stop=True)
                    nc.vector.reciprocal_approx_fast(
                        bc_sb[0:64, hi * 512:(hi + 1) * 512],
                        ps_bc[0:64, :])
                nc.vector.tensor_mul(
                    otn_sb[p][0:64, qs], ps_o[0:64, 0:512],
                    bc_sb[0:64, 0:512])
                nc.vector.tensor_mul(
                    otnB[0:64, qs], ps_o[0:64, 512:1024],
                    bc_sb[0:64, 512:1024])
                nc.sync.dma_start(otn_sb[p][64:128, qs], otnB[0:64, qs])
                if p == NPAIR - 1:
                    for i in range(4 * qc, min(4 * qc + 4, TT)):
                        for ch in range(C // 512):
                            fpush(10 ** 9,
                                  lambda i=i, ch=ch: emit_yproj_unit(i, ch))

            seq = [(p, qc) for p in range(NPAIR) for qc in range(QC)]
            otnB_t = [nrm_pool.tile([64, t], BF, tag="otnB", name=f"otnB{p}")
                      for p in range(NPAIR)]
            # preseed the very first chunk's first two key tiles
            pend_next = [emit_s_exp(0, 0, 0), emit_s_exp(0, 0, 1)]

            for idx, (p, qc) in enumerate(seq):
                # pair 0's early chunks have little attention work per
                # iteration; pay out more projection units there
                fn = 3 if p == 0 else (2 if p == NPAIR - 1 else 1)
                if qc + 1 < QC and (p == 0 or qc > 0):
                    # chunk qc+1 for this pair (chunk 1 of pairs >0 was
                    # already queued by the previous pair)
                    fpush(idx + 1, lambda p=p, tc_=qc + 1: emit_qk_unit(p, 0, tc_))
                    fpush(idx + 1, lambda p=p, tc_=qc + 1: emit_qk_unit(p, 1, tc_))
                if p == 0 and qc + 1 < QC:
                    for i in range(4 * (qc + 1), min(4 * (qc + 2), TT)):
                        fpush(idx + 1, lambda i=i: emit_v_unit(i))
                if p + 1 < NPAIR and qc == max(0, QC - 2):
                    nidx = (p + 1) * QC
                    fpush(nidx, lambda p=p: emit_qk_unit(p + 1, 0, 0))
                    fpush(nidx, lambda p=p: emit_qk_unit(p + 1, 1, 0))
                    if QC > 1:
                        fpush(nidx + 1, lambda p=p: emit_qk_unit(p + 1, 0, 1))
                        fpush(nidx + 1, lambda p=p: emit_qk_unit(p + 1, 1, 1))

                ktmax = 4 * qc + 3
                ps_o = psO.tile([128, 1024], F32, tag="O", name=f"o{p}_{qc}")
                pend = deque(pend_next)
                pend_next = []
                for ktp in range(2, ktmax + 1, 2):
                    pend.append(emit_s_exp(p, qc, ktp))
                    pend.append(emit_s_exp(p, qc, ktp + 1))
                    while len(pend) > 4:
                        emit_ot(p, pend.popleft(), ps_o, ktmax)
                    if ktp < ktmax - 1:
                        feed(fn)
                # pre-emit the next chunk's first scores/exp so ACT keeps
                # running across the boundary while we drain and normalize
                if idx + 1 < len(seq):
                    np_, nqc = seq[idx + 1]
                    force_feed(idx + 1)
                    pend_next = [emit_s_exp(np_, nqc, 0), emit_s_exp(np_, nqc, 1)]
                while pend:
                    emit_ot(p, pend.popleft(), ps_o, ktmax)
                normalize(p, qc, ps_o, otnB_t[p])
                feed(fn)
            feed(len(feeder))

    nc.compile()
    return nc


def get_nc(t=T):
    if t not in _CACHE:
        _CACHE[t] = build(t)
    return _CACHE[t]


def make_masks():
    r = np.arange(128)[:, None]
    c = np.arange(128)[None, :]
    return (r <= c).astype(BF_NP)


def prep_inputs(x, W_attn, b_attn, W_proj, t=T):
    """Per-core input maps. Core c: batch c//2, head group c%2."""
    masks = make_masks()
    qs, ks, vs = W_attn[:, :C], W_attn[:, C:2 * C], W_attn[:, 2 * C:]
    bqs = b_attn[:C]
    QCt = t // 512
    in_maps = []
    for core in range(N_CORES):
        b, hg = core // 2, core % 2
        xb = x[b, :t].astype(np.float32)
        xTb = np.ascontiguousarray(
            xb.T.reshape(C, QCt, 512).transpose(1, 0, 2)).astype(BF_NP)
        # x8: [QC, KC//2, 128, cg*512+t] fp8
        x8 = np.ascontiguousarray(
            xb.astype(FP8_NP).astype(np.float32).T
            .reshape(KC // 2, 2, 128, QCt, 512).transpose(3, 0, 2, 1, 4)
            .reshape(QCt, KC // 2, 128, 1024)).astype(FP8_NP)
        # wqk: [NPAIR, which, KC//2, p, cg*128 + m] fp8, x32
        wqk_arr = np.zeros((NPAIR, 2, KC // 2, 128, 2, 128), dtype=FP8_NP)
        bqcols = []
        for p in range(NPAIR):
            ha = hg * HPC + 2 * p
            for which, wsrc in enumerate((qs, ks)):
                blk = wsrc[:, ha * D:(ha + 2) * D] * W_SCALE   # [C, 128]
                wqk_arr[p, which] = blk.reshape(
                    KC // 2, 2, 128, 128).transpose(0, 2, 1, 3).astype(FP8_NP)
            bqcols.append(bqs[ha * D:(ha + 2) * D] * W_SCALE)
        wqk = np.ascontiguousarray(
            wqk_arr.reshape(NPAIR, 2, KC // 2, 128, 256))
        wv = vs[:, hg * HPC * D:(hg + 1) * HPC * D].astype(BF_NP)
        wp = W_proj[hg * HPC * D:(hg + 1) * HPC * D, :].astype(BF_NP)
        bq = np.stack(bqcols, axis=1).astype(np.float32)
        in_maps.append({"xT": xTb, "x8": x8, "wqk": wqk, "wv": wv, "wp": wp,
                        "bq": bq, "masks": masks})
    return in_maps


def kernel(x, W_attn, b_attn, W_proj, b_proj):
    x = np.asarray(x, dtype=np.float32)
    W_attn = np.asarray(W_attn, dtype=np.float32)
    b_attn = np.asarray(b_attn, dtype=np.float32)
    W_proj = np.asarray(W_proj, dtype=np.float32)
    b_proj = np.asarray(b_proj, dtype=np.float32)

    nc = get_nc(T)
    in_maps = prep_inputs(x, W_attn, b_attn, W_proj, T)
    res = run_bass_kernel_spmd(nc, in_maps, list(range(N_CORES)))
    # host reduction: sum the two head-group partials; bias terms
    bias = b_attn[2 * C:] @ W_proj + b_proj  # v-bias passes through linearly
    y = np.empty((B, T, C), dtype=np.float32)
    for b in range(B):
        y[b] = (res.results[2 * b]["y"].astype(np.float32)
                + res.results[2 * b + 1]["y"].astype(np.float32) + bias)
    return y



# revision 55
# speedup vs baseline: 1.0095x; 1.0080x over previous
"""Causal self-attention kernel for 8 Trainium2 NeuronCores.

Sharding: core c handles batch b = c//2 and head-group hg = c%2 (8 of 16
heads). Each core computes qkv projection for its heads, head-parallel
causal attention, and a partial output projection; the host sums the two
head-group partials per batch and adds the bias terms.

The Q/K projections run in fp8 (e4m3) DoubleRow perf mode: x and the
q/k weight blocks are quantized to fp8 (weights pre-scaled x32 so fp8
stays in normal range) and the contraction C=1024 is packed as 4
k-tiles of 256 (2 sub-tiles per partition), halving the projection
matmul count. The exp activation folds the compensating 2^-13 scale
(= 0.125 softmax scale / 32^2). Softmax averaging absorbs the ~4% fp8
noise (~1.1e-2 rel total); V/P/otn/wp stay bf16 - fp8 noise on any
tensor feeding the 512-long output contraction alone costs ~2e-2.

Schedule: all projection work (QK chains, V chains, output projection)
is decomposed into small units fed one-per-iteration into the attention
loop so the PE (the bottleneck engine, ~82% busy) never runs dry.
Softmax normalization for chunk qc is emitted after chunk qc+1's first
scores so neither PE nor ACT stalls at boundaries.

On-chip layouts (per core):
  xT  [C, T] as 4 column-chunks [128,512] x 8 k-tiles (bf16, for V)
  x8  fp8 copy of x, paired k-tiles [128, 2(cg), 512] x 4 (for QK-DR)
  Q^T/K^T per pair    [128 = headA dims | headB dims, T] (bf16)
  V   per token tile  [128, 8*65] - per-head stripes [V_h | ones] so the
                      P@V matmul's ones column accumulates softmax sums
  S^T per (kt,qc)     [keys=128, queries<=512] in PSUM; exp on ScalarE
  O^T per (pair,qc)   [128,1024] PSUM: head A cols 0:512, head B 512:1024;
                      row 64 of each half = softmax sums
  softmax denominators broadcast across partitions via K=1 f32r matmul
"""
import sys
sys.path.insert(0, '/opt/trn_rl_repo')
from collections import deque
import numpy as np
import ml_dtypes

from concourse import bacc, mybir
import concourse.tile as tile
from concourse.bass_utils import run_bass_kernel_spmd

B, T, C, H = 4, 2048, 1024, 16
D = C // H           # 64
HPC = H // 2         # 8 heads per core
NPAIR = HPC // 2     # 4
N_CORES = 8
KC = C // 128        # 8 contraction tiles for projections
BF = mybir.dt.bfloat16
F32 = mybir.dt.float32
F32R = mybir.dt.float32r
I32 = mybir.dt.int32
FP8 = mybir.dt.float8e4
DR = mybir.MatmulPerfMode.DoubleRow
FP8_NP = ml_dtypes.float8_e4m3
EXPSCALE = float(2.0 ** -13)
W_SCALE = 32.0
SCH_A = float((1 << 23) / np.log(2))
SCH_B = float(127 * (1 << 23) - int(0.0430 * (1 << 23)))
USE_SCHRAUDOLPH = False
S_PERF_MODE = None
QK_DT = mybir.dt.bfloat16
BF_NP = ml_dtypes.bfloat16

_CACHE = {}


def build(t=T):
    QC = t // 512        # query chunks
    TT = t // 128        # token/key tiles
    nc = bacc.Bacc("TRN2", target_bir_lowering=False, debug=False,
                   num_devices=N_CORES)
    xTd = nc.dram_tensor("xT", [QC, C, 512], BF, kind="ExternalInput").ap()
    x8d = nc.dram_tensor("x8", [QC, KC // 2, 128, 1024], FP8,
                         kind="ExternalInput").ap()
    wqk = nc.dram_tensor("wqk", [NPAIR, 2, KC // 2, 128, 256], FP8,
                         kind="ExternalInput").ap()
    wv = nc.dram_tensor("wv", [C, HPC * D], BF, kind="ExternalInput").ap()
    wp = nc.dram_tensor("wp", [HPC * D, C], BF, kind="ExternalInput").ap()
    bq = nc.dram_tensor("bq", [128, NPAIR], F32, kind="ExternalInput").ap()
    masks = nc.dram_tensor("masks", [128, 128], BF, kind="ExternalInput").ap()
    y = nc.dram_tensor("y", [t, C], BF, kind="ExternalOutput").ap()

    with tile.TileContext(nc) as tc:
        with tc.tile_pool(name="const", bufs=1) as cpool, \
             tc.tile_pool(name="work", bufs=1) as wpool, \
             tc.tile_pool(name="psS", bufs=2, space="PSUM") as psS, \
             tc.tile_pool(name="psO", bufs=1, space="PSUM") as psO, \
             tc.tile_pool(name="psC", bufs=2, space="PSUM") as psC, \
             tc.tile_pool(name="att", bufs=12) as att_pool, \
             tc.tile_pool(name="nrm", bufs=2) as nrm_pool, \
             tc.tile_pool(name="bits", bufs=2) as bits_pool, \
             tc.tile_pool(name="yo", bufs=3) as y_pool:

            # ---- tiny constants first (vector-engine DMA queue)
            mask_sb = cpool.tile([128, 128], BF, tag="mask")
            nc.sync.dma_start(mask_sb[:], masks)
            bq_sb = cpool.tile([128, NPAIR], F32, tag="bq")
            nc.sync.dma_start(bq_sb[:], bq)
            ones32 = cpool.tile([128, 64], F32, tag="ones32")
            nc.vector.memset(ones32[:], 1.0)
            ones_fr = cpool.tile([128, 64], F32R, tag="ones_fr")
            nc.vector.tensor_copy(ones_fr[:], ones32[:])

            # ---- weights / activations DMA, critical-path order:
            # pair0 wqk block + xT chunk 0 + wv, then the rest, wp last
            wqk_sb = {}
            xT_sb = {}
            x8_sb = {}
            wv_sb = []
            tl = cpool.tile([128, 2, KC // 2, 256], FP8, tag="wqk0",
                            name="wqk0")
            nc.scalar.dma_start(
                tl[:], wqk[0].rearrange("w k p m -> p w k m"))
            wqk_sb[0] = tl
            for k in range(KC // 2):
                tl = cpool.tile([128, 2, 512], FP8, tag=f"x8{k}_0",
                                name=f"x8{k}_0")
                eng = nc.sync if k % 2 == 0 else nc.scalar
                eng.dma_start(
                    tl[:], x8d[0, k].rearrange("p (c t) -> p c t", c=2))
                x8_sb[(k, 0)] = tl
            for k in range(KC):
                tl = cpool.tile([128, 512], BF, tag=f"xT{k}_0", name=f"xT{k}_0")
                eng = nc.sync if k % 2 == 0 else nc.gpsimd
                eng.dma_start(tl[:], xTd[0, k * 128:(k + 1) * 128, :])
                xT_sb[(k, 0)] = tl
            # x8 chunk 1 ahead of wv: qk chunk-1 units gate the (0,1)
            # attention chunk earlier than the wv-consuming PV does
            if QC > 1:
                for k in range(KC // 2):
                    tl = cpool.tile([128, 2, 512], FP8, tag=f"x8{k}_1",
                                    name=f"x8{k}_1")
                    eng = nc.scalar if k % 2 == 0 else nc.sync
                    eng.dma_start(
                        tl[:], x8d[1, k].rearrange("p (c t) -> p c t", c=2))
                    x8_sb[(k, 1)] = tl
            for k in range(KC):
                tl = cpool.tile([128, HPC * D], BF, tag=f"wv{k}", name=f"wv{k}")
                nc.scalar.dma_start(tl[:], wv[k * 128:(k + 1) * 128, :])
                wv_sb.append(tl)
            for p in range(1, NPAIR):
                tl = cpool.tile([128, 2, KC // 2, 256], FP8, tag=f"wqk{p}",
                                name=f"wqk{p}")
                nc.gpsimd.dma_start(
                    tl[:], wqk[p].rearrange("w k p m -> p w k m"))
                wqk_sb[p] = tl
            for tch in range(1, QC):
                for k in range(KC // 2):
                    if (k, tch) in x8_sb:
                        continue
                    tl = cpool.tile([128, 2, 512], FP8, tag=f"x8{k}_{tch}",
                                    name=f"x8{k}_{tch}")
                    eng = nc.scalar if k % 2 == 0 else nc.sync
                    eng.dma_start(
                        tl[:], x8d[tch, k].rearrange("p (c t) -> p c t", c=2))
                    x8_sb[(k, tch)] = tl
                for k in range(KC):
                    tl = cpool.tile([128, 512], BF, tag=f"xT{k}_{tch}",
                                    name=f"xT{k}_{tch}")
                    eng = nc.sync if k % 2 == 0 else nc.gpsimd
                    eng.dma_start(tl[:], xTd[tch, k * 128:(k + 1) * 128, :])
                    xT_sb[(k, tch)] = tl
            wp_sb = []
            for p in range(NPAIR):
                tl = cpool.tile([128, C], BF, tag=f"wp{p}", name=f"wp{p}")
                nc.scalar.dma_start(tl[:], wp[p * 128:(p + 1) * 128, :])
                wp_sb.append(tl)

            # persistent intermediates
            qt_sb = [wpool.tile([128, t], QK_DT, tag=f"qt{p}", name=f"qt{p}")
                     for p in range(NPAIR)]
            kt_sb = [wpool.tile([128, t], QK_DT, tag=f"kt{p}", name=f"kt{p}")
                     for p in range(NPAIR)]
            v_sb = [wpool.tile([128, HPC * 65], BF, tag=f"v{i}", name=f"v{i}")
                    for i in range(TT)]
            otn_sb = [wpool.tile([128, t], BF, tag=f"otn{p}", name=f"otn{p}")
                      for p in range(NPAIR)]

            # ---- chain units (projection work fed into the attention loop)
            def emit_qk_unit(p, which, tch):
                # which: 0=q, 1=k ; writes qt/kt chunk tch for pair p.
                # fp8 DoubleRow: 4 k-tiles of 256 contraction.
                ps = psC.tile([128, 512], F32, tag="c",
                              name=f"qk{p}_{which}_{tch}")
                for k in range(KC // 2):
                    nc.tensor.matmul(
                        ps[:],
                        wqk_sb[p][:, which, k]
                        .rearrange("p (c m) -> p c m", c=2),
                        x8_sb[(k, tch)][:],
                        start=(k == 0), stop=(k == KC // 2 - 1), perf_mode=DR)
                dst = (qt_sb if which == 0 else kt_sb)[p]
                if which == 0:
                    nc.vector.tensor_scalar_add(
                        dst[:, tch * 512:(tch + 1) * 512], ps[:],
                        bq_sb[:, p:p + 1])
                else:
                    nc.vector.tensor_copy(
                        dst[:, tch * 512:(tch + 1) * 512], ps[:])

            def emit_v_unit(i):
                ps = psC.tile([128, 512], F32, tag="c", name=f"v{i}")
                tch, j = divmod(i, 4)
                for k in range(KC):
                    nc.tensor.matmul(
                        ps[:, 0:HPC * D],
                        xT_sb[(k, tch)][:, j * 128:(j + 1) * 128],
                        wv_sb[k][:], start=(k == 0), stop=(k == KC - 1))
                vt = v_sb[i][:].rearrange("p (h e) -> p h e", e=65)
                nc.vector.memset(vt[:, :, 64:65], 1.0)
                nc.vector.tensor_copy(
                    vt[:, :, 0:64],
                    ps[:, 0:HPC * D].rearrange("p (h d) -> p h d", d=64))

            def emit_yproj_unit(i, ch):
                ps = psC.tile([128, 512], F32, tag="c", name=f"y{i}_{ch}")
                for p in range(NPAIR):
                    nc.tensor.matmul(
                        ps[:], otn_sb[p][:, i * 128:(i + 1) * 128],
                        wp_sb[p][:, ch * 512:(ch + 1) * 512],
                        start=(p == 0), stop=(p == NPAIR - 1))
                ysb = y_pool.tile([128, 512], BF, tag="ysb", name=f"ysb{i}_{ch}")
                nc.vector.tensor_copy(ysb[:], ps[:])
                nc.sync.dma_start(
                    y[i * 128:(i + 1) * 128, ch * 512:(ch + 1) * 512], ysb[:])

            feeder = []   # entries: (deadline_seq_idx, order, callable)
            _fcnt = [0]

            def fpush(dl, fn):
                # keep sorted by (deadline, insertion order) so hard
                # deadlines are never stuck behind unbounded (yproj) work
                import bisect
                _fcnt[0] += 1
                bisect.insort(feeder, (dl, _fcnt[0], fn))

            def feed(n):
                for _ in range(n):
                    if not feeder:
                        return
                    feeder.pop(0)[2]()

            def force_feed(idx_limit):
                # emit every unit that must precede seq[idx_limit]
                while feeder and feeder[0][0] <= idx_limit:
                    feeder.pop(0)[2]()

            # ---- prologue: first chains so PE starts as soon as DMA lands
            emit_qk_unit(0, 0, 0)
            emit_qk_unit(0, 1, 0)
            for i in range(min(4, TT)):
                emit_v_unit(i)

            def emit_ot(p, pend_item, ps_o, ktmax):
                kt, lo, pt = pend_item
                for hi in (0, 1):
                    h = 2 * p + hi
                    nc.tensor.matmul(
                        ps_o[0:65, hi * 512 + lo:hi * 512 + 512],
                        v_sb[kt][:, 65 * h:65 * h + 65],
                        pt[:, hi * 512 + lo:(hi + 1) * 512],
                        start=(kt == 0), stop=(kt == ktmax))

            def emit_s_exp(p, qc, kt):
                delta = kt - 4 * qc
                lo = 128 * delta if delta > 0 else 0
                ps_s = psS.tile([128, 1024], F32, tag="S",
                                name=f"s{p}_{qc}_{kt}")
                for hi, (p0, p1) in enumerate(((0, 64), (64, 128))):
                    nc.tensor.matmul(
                        ps_s[:, hi * 512 + lo:(hi + 1) * 512],
                        kt_sb[p][p0:p1, kt * 128:(kt + 1) * 128],
                        qt_sb[p][p0:p1, qc * 512 + lo:(qc + 1) * 512],
                        start=True, stop=True, perf_mode=S_PERF_MODE)
                pt = att_pool.tile([128, 1024], BF, tag="pt",
                                   name=f"pt{p}_{qc}_{kt}")
                if USE_SCHRAUDOLPH and p >= 1 and qc >= 2 and delta < 0 and kt % 4 == 3:
                    # Schraudolph exp on DVE to relieve ACT
                    bits = bits_pool.tile([128, 1024], I32, tag="bits",
                                          name=f"bits{p}_{qc}_{kt}")
                    nc.vector.tensor_scalar(
                        bits[:], ps_s[:], SCH_A, SCH_B,
                        op0=mybir.AluOpType.mult,
                        op1=mybir.AluOpType.add)
                    nc.vector.tensor_copy(pt[:], bits[:].bitcast(F32))
                else:
                    nc.scalar.activation(
                        pt[:].rearrange("p (u c) -> p u c", u=2)[:, :, lo:512],
                        ps_s[:].rearrange("p (u c) -> p u c", u=2)[:, :, lo:512],
                        mybir.ActivationFunctionType.Exp, scale=EXPSCALE)
                if delta >= 0:
                    # only cols [lo, lo+128) straddle the causal boundary
                    for hi in range(2):
                        sl = pt[:, hi * 512 + lo:hi * 512 + lo + 128]
                        nc.vector.tensor_mul(sl, sl, mask_sb[:])
                return (kt, lo, pt)

            def normalize(p, qc, ps_o, otnB):
                qs = slice(qc * 512, (qc + 1) * 512)
                sums_fr = nrm_pool.tile([128, 1024], F32R, tag="sums",
                                        name=f"sums{p}_{qc}")
                nc.vector.tensor_copy(sums_fr[64:65, 0:1024],
                                      ps_o[64:65, 0:1024])
                bc_sb = nrm_pool.tile([64, 1024], F32, tag="bc",
                                      name=f"bc{p}_{qc}")
                for hi in (0, 1):
                    ps_bc = psC.tile([128, 512], F32, tag="c",
                                     name=f"bc{p}_{qc}_{hi}")
                    nc.tensor.matmul(
                        ps_bc[0:64, :], ones_fr[64:65, 0:64],
                        sums_fr[64:65, hi * 512:(hi + 1) * 512],
                        start=True, stop=True)
                    nc.vector.reciprocal_approx_fast(
                        bc_sb[0:64, hi * 512:(hi + 1) * 512],
                        ps_bc[0:64, :])
                nc.vector.tensor_mul(
                    otn_sb[p][0:64, qs], ps_o[0:64, 0:512],
                    bc_sb[0:64, 0:512])
                nc.vector.tensor_mul(
                    otnB[0:64, qs], ps_o[0:64, 512:1024],
                    bc_sb[0:64, 512:1024])
                nc.sync.dma_start(otn_sb[p][64:128, qs], otnB[0:64, qs])
                if p == NPAIR - 1:
                    for i in range(4 * qc, min(4 * qc + 4, TT)):
                        for ch in range(C // 512):
                            fpush(10 ** 9,
                                  lambda i=i, ch=ch: emit_yproj_unit(i, ch))

            seq = [(p, qc) for p in range(NPAIR) for qc in range(QC)]
            otnB_t = [nrm_pool.tile([64, t], BF, tag="otnB", name=f"otnB{p}")
                      for p in range(NPAIR)]
            # preseed the very first chunk's first two key tiles
            pend_next = [emit_s_exp(0, 0, 0), emit_s_exp(0, 0, 1)]

            for idx, (p, qc) in enumerate(seq):
                # pair 0's early chunks have little attention work per
                # iteration; pay out more projection units there
                fn = 3 if p == 0 else (2 if p == NPAIR - 1 else 1)
                if qc + 1 < QC and (p == 0 or qc > 0):
                    # chunk qc+1 for this pair (chunk 1 of pairs >0 was
                    # already queued by the previous pair)
                    fpush(idx + 1, lambda p=p, tc_=qc + 1: emit_qk_unit(p, 0, tc_))
                    fpush(idx + 1, lambda p=p, tc_=qc + 1: emit_qk_unit(p, 1, tc_))
                if p == 0 and qc + 1 < QC:
                    for i in range(4 * (qc + 1), min(4 * (qc + 2), TT)):
                        fpush(idx + 1, lambda i=i: emit_v_unit(i))
                if p + 1 < NPAIR and qc == max(0, QC - 2):
                    nidx = (p + 1) * QC
                    fpush(nidx, lambda p=p: emit_qk_unit(p + 1, 0, 0))
                    fpush(nidx, lambda p=p: emit_qk_unit(p + 1, 1, 0))
                    if QC > 1:
                        fpush(nidx + 1, lambda p=p: emit_qk_unit(p + 1, 0, 1))
                        fpush(nidx + 1, lambda p=p: emit_qk_unit(p + 1, 1, 1))

                ktmax = 4 * qc + 3
                ps_o = psO.tile([128, 1024], F32, tag="O", name=f"o{p}_{qc}")
                pend = deque(pend_next)
                pend_next = []
                for ktp in range(2, ktmax + 1, 2):
                    pend.append(emit_s_exp(p, qc, ktp))
                    pend.append(emit_s_exp(p, qc, ktp + 1))
                    while len(pend) > 4:
                        emit_ot(p, pend.popleft(), ps_o, ktmax)
                    if ktp < ktmax - 1:
                        feed(fn)
                # pre-emit the next chunk's first scores/exp so ACT keeps
                # running across the boundary while we drain and normalize
                if idx + 1 < len(seq):
                    np_, nqc = seq[idx + 1]
                    force_feed(idx + 1)
                    pend_next = [emit_s_exp(np_, nqc, 0), emit_s_exp(np_, nqc, 1)]
                while pend:
                    emit_ot(p, pend.popleft(), ps_o, ktmax)
                normalize(p, qc, ps_o, otnB_t[p])
                feed(fn)
            feed(len(feeder))

    nc.compile()
    return nc


def get_nc(t=T):
    if t not in _CACHE:
        _CACHE[t] = build(t)
    return _CACHE[t]


def make_masks():
    r = np.arange(128)[:, None]
    c = np.arange(128)[None, :]
    return (r <= c).astype(BF_NP)


def prep_inputs(x, W_attn, b_attn, W_proj, t=T):
    """Per-core input maps. Core c: batch c//2, head group c%2."""
    masks = make_masks()
    qs, ks, vs = W_attn[:, :C], W_attn[:, C:2 * C], W_attn[:, 2 * C:]
    bqs = b_attn[:C]
    QCt = t // 512
    in_maps = []
    for core in range(N_CORES):
        b, hg = core // 2, core % 2
        xb = x[b, :t].astype(np.float32)
        xTb = np.ascontiguousarray(
            xb.T.reshape(C, QCt, 512).transpose(1, 0, 2)).astype(BF_NP)
        # x8: [QC, KC//2, 128, cg*512+t] fp8
        x8 = np.ascontiguousarray(
            xb.astype(FP8_NP).astype(np.float32).T
            .reshape(KC // 2, 2, 128, QCt, 512).transpose(3, 0, 2, 1, 4)
            .reshape(QCt, KC // 2, 128, 1024)).astype(FP8_NP)
        # wqk: [NPAIR, which, KC//2, p, cg*128 + m] fp8, x32
        wqk_arr = np.zeros((NPAIR, 2, KC // 2, 128, 2, 128), dtype=FP8_NP)
        bqcols = []
        for p in range(NPAIR):
            ha = hg * HPC + 2 * p
            for which, wsrc in enumerate((qs, ks)):
                blk = wsrc[:, ha * D:(ha + 2) * D] * W_SCALE   # [C, 128]
                wqk_arr[p, which] = blk.reshape(
                    KC // 2, 2, 128, 128).transpose(0, 2, 1, 3).astype(FP8_NP)
            bqcols.append(bqs[ha * D:(ha + 2) * D] * W_SCALE)
        wqk = np.ascontiguousarray(
            wqk_arr.reshape(NPAIR, 2, KC // 2, 128, 256))
        wv = vs[:, hg * HPC * D:(hg + 1) * HPC * D].astype(BF_NP)
        wp = W_proj[hg * HPC * D:(hg + 1) * HPC * D, :].astype(BF_NP)
        bq = np.stack(bqcols, axis=1).astype(np.float32)
        in_maps.append({"xT": xTb, "x8": x8, "wqk": wqk, "wv": wv, "wp": wp,
                        "bq": bq, "masks": masks})
    return in_maps


def kernel(x, W_attn, b_attn, W_proj, b_proj):
    x = np.asarray(x, dtype=np.float32)
    W_attn = np.asarray(W_attn, dtype=np.float32)
    b_attn = np.asarray(b_attn, dtype=np.float32)
    W_proj = np.asarray(W_proj, dtype=np.float32)
    b_proj = np.asarray(b_proj, dtype=np.float32)

    nc = get_nc(T)
    in_maps = prep_inputs(x, W_attn, b_attn, W_proj, T)
    res = run_bass_kernel_spmd(nc, in_maps, list(range(N_CORES)))
    # host reduction: sum the two head-group partials; bias terms
    bias = b_attn[2 * C:] @ W_proj + b_proj  # v-bias passes through linearly
    y = np.empty((B, T, C), dtype=np.float32)
    for b in range(B):
        y[b] = (res.results[2 * b]["y"].astype(np.float32)
                + res.results[2 * b + 1]["y"].astype(np.float32) + bias)
    return y

